# revision 49
# baseline (speedup 1.0000x reference)
"""Trainium2 Bass kernel for nn_H_DYNA_42348377538865 (scatter_memory GRU + memory attention).

Self-contained: shards node dim N=512 across 8 NeuronCores (64 nodes/core),
runs a fully-unrolled 24-step recurrence per core, gathers on host.

Layout: feature-on-partitions, (node, batch) on free dim (col = n_local*32 + b,
NB=2048 cols/core, 4 column chunks of 512). Design notes (~2.2x vs the v1
scatter kernel's 634us; TimelineSim 291us, all four engines ~80% busy):
  - single activation table: sigmoid computed as 0.5+0.5*tanh(x/2) so the
    Scalar engine only ever uses {Exp, Tanh, Copy/Identity} -> zero
    LoadActFuncSet swaps in steady state (each swap costs 1283ns; the v1
    kernel spent 164us on them).
  - biases folded into matmuls via a const-1.0 row of the state tiles;
    encode inputs x_0..x_11 pre-staged as extra state rows with per-step
    weight rows selecting them (no per-step x DMA, no step barriers).
  - decode x-feedback (x_t = y_{t-1}) folded algebraically into the decode
    weights (rank-1 update W + ow@Wx^T; candidate path gets an extra wxd
    matmul); y itself is never computed on device - decode h states are
    DMA'd out and y = ow@h + bo applied on host.
  - gates: ONE combined [*,128] z|r matmul + ONE tanh. The zr matmul for
    step t+1 and its tanh are issued during step t as a split pair
    (W^T h_old + W^T t1, exploiting linearity), and q(h_new) is likewise
    Wq^T h_old + Wq^T t1, so the in-place h update runs OFF the recurrence
    critical path on the otherwise-idle GPSIMD engine.
  - engine balance (each ~240us busy): ACT: exp, gate-tanh, cand-tanh,
    q-copy; DVE: recip, softmax-normalize mul, r/z gate affines (4x-mode
    tensor_scalar, the r one partition-crossed 64->0), hc-h sub, z*dl mul;
    GPSIMD: r*h product and the h update (both off the critical cycle);
    PE: 10-11 streams/chunk.
  - hardware rules learned (see test scripts): multi-tensor DVE/GPSIMD ops
    need all operands at the same start partition; single-input ops
    (activation, tensor_copy, tensor_scalar) may cross partition offsets,
    but reciprocal_approx_fast silently NaNs on crossed reads; tensor_tensor
    divide and fused scalar_tensor_tensor on GPSIMD are rejected/crash.
  - rolling q-cache as v1: 12 slots in 3x[128,NB] tiles; slot j pairs with
    memory slice s=(j-t)%12 via 12 precomputed rotation stacks.
"""
import numpy as np
import sys

for _p in ("/opt/trn_rl_repo",):
    if _p not in sys.path:
        sys.path.append(_p)

import concourse.bass as bass
import concourse.bacc as bacc
import concourse.mybir as mybir
import concourse.tile as tile
from concourse import bass_utils

B, T, HORIZON, N = 32, 12, 12, 512
IN, OUT, H, P = 1, 1, 64, 32
S, ML, MG, DE = 12, 64, 32, 10
NCORES = 8
NL = N // NCORES        # 64
NB = NL * B             # 2048
NSTEP = T + HORIZON     # 24
CH = 4                  # column chunks
CW = NB // CH           # 512

F32 = mybir.dt.float32
BF16 = mybir.dt.bfloat16
AF = mybir.ActivationFunctionType
ALU = mybir.AluOpType


def build_nc():
    nc = bacc.Bacc("TRN2", target_bir_lowering=False, debug=False)
    d = {}
    d["hxinit"] = nc.dram_tensor("hxinit", [77, NB], BF16, kind="ExternalInput")
    d["rhpinit"] = nc.dram_tensor("rhpinit", [128, NB], BF16, kind="ExternalInput")
    d["qbinit"] = nc.dram_tensor("qbinit", [128, NB], BF16, kind="ExternalInput")
    d["memstack"] = nc.dram_tensor("memstack", [128, S * 3 * 96], BF16, kind="ExternalInput")
    d["nsw"] = nc.dram_tensor("nsw", [64, NL * 64], BF16, kind="ExternalInput")
    d["fmean"] = nc.dram_tensor("fmean", [96, 64], BF16, kind="ExternalInput")
    d["fsum"] = nc.dram_tensor("fsum", [96, 64], BF16, kind="ExternalInput")
    d["zrwa"] = nc.dram_tensor("zrwa", [77, T * 128], BF16, kind="ExternalInput")
    d["zrwd"] = nc.dram_tensor("zrwd", [65, 128], BF16, kind="ExternalInput")
    d["cwa"] = nc.dram_tensor("cwa", [128, T * 64], BF16, kind="ExternalInput")
    d["cwd"] = nc.dram_tensor("cwd", [128, 64], BF16, kind="ExternalInput")
    d["wxd"] = nc.dram_tensor("wxd", [65, 64], BF16, kind="ExternalInput")
    d["qw2"] = nc.dram_tensor("qw2", [65, 32], BF16, kind="ExternalInput")
    hs_d = nc.dram_tensor("hsave", [HORIZON * 64, NB], BF16, kind="ExternalOutput")

    with tile.TileContext(nc) as tc:
        with (
            tc.tile_pool(name="consts", bufs=1) as cp,
            tc.tile_pool(name="sp", bufs=8) as sp,
            tc.tile_pool(name="szp", bufs=12) as szp,
            tc.tile_pool(name="pp_lg", bufs=2, space="PSUM") as pp_lg,
            tc.tile_pool(name="pp_fu", bufs=1, space="PSUM") as pp_fu,
            tc.tile_pool(name="pp_su", bufs=1, space="PSUM") as pp_su,
            tc.tile_pool(name="pp_zr", bufs=2, space="PSUM") as pp_zr,
            tc.tile_pool(name="pp_acc", bufs=1, space="PSUM") as pp_acc,
            tc.tile_pool(name="pp_qp", bufs=1, space="PSUM") as pp_qp,
        ):
            # critical-path loads first: initial state, zr weights, q-cache
            # and memstack gate step 0; the rest arrives under compute
            hx = cp.tile([77, NB], BF16)
            nc.sync.dma_start(hx[:], d["hxinit"].ap())
            zrwa = cp.tile([77, T * 128], BF16)
            nc.sync.dma_start(zrwa[:], d["zrwa"].ap())
            qb = []
            for g in range(3):
                q = cp.tile([128, NB], BF16, name=f"qb{g}")
                nc.sync.dma_start(q[:], d["qbinit"].ap())
                qb.append(q)
            msk = cp.tile([128, S * 3 * 96], BF16)
            nc.sync.dma_start(msk[:], d["memstack"].ap())
            fmean = cp.tile([96, 64], BF16)
            nc.sync.dma_start(fmean[:], d["fmean"].ap())
            fsum = cp.tile([96, 64], BF16)
            nc.sync.dma_start(fsum[:], d["fsum"].ap())
            nsw = cp.tile([64, NL * 64], BF16)
            nc.sync.dma_start(nsw[:], d["nsw"].ap())
            # rhp: row 0 const 1.0, rows 1:13 x, rows 64:128 m1 = r*h
            rhp = cp.tile([128, NB], BF16)
            nc.sync.dma_start(rhp[:], d["rhpinit"].ap())
            # h duplicate at partitions 64:128 (rows 64:128 of rhpinit are 0)
            hp = cp.tile([128, NB], BF16)
            nc.sync.dma_start(hp[:], d["rhpinit"].ap())
            cwa = cp.tile([128, T * 64], BF16)
            nc.sync.dma_start(cwa[:], d["cwa"].ap())
            qw2 = cp.tile([65, 32], BF16)
            nc.sync.dma_start(qw2[:], d["qw2"].ap())
            zrwd = cp.tile([65, 128], BF16)
            nc.sync.dma_start(zrwd[:], d["zrwd"].ap())
            cwd = cp.tile([128, 64], BF16)
            nc.sync.dma_start(cwd[:], d["cwd"].ap())
            wxd = cp.tile([65, 64], BF16)
            nc.sync.dma_start(wxd[:], d["wxd"].ap())

            def zr_w(t):
                # (zr weights AP, stream row count) for step t
                if t < T:
                    return zrwa[:, t * 128 : (t + 1) * 128], 77
                if t == T:  # decode step 0 reuses x = xs[T-1]
                    return zrwa[:, (T - 1) * 128 : T * 128], 77
                return zrwd[:], 65

            # zr(0)+tanh for all chunks from the initial state
            szr_pend = []
            for c in range(CH):
                zw0, nr0 = zr_w(0)
                zrt = pp_zr.tile([128, CW], F32, tag="zr", name=f"zr0_{c}")
                nc.tensor.matmul(
                    zrt[:], zw0, hx[0:nr0, c * CW : (c + 1) * CW],
                    start=True, stop=True,
                )
                szt = szp.tile([128, CW], BF16, tag="szr", name=f"szr0_{c}")
                nc.scalar.activation(szt[:], zrt[:], AF.Tanh, scale=0.5)
                szr_pend.append(szt)

            for t in range(NSTEP):
                r = t % S
                j = t % S
                g_w, row_w = j // 4, (j % 4) * 32
                dec = t >= T
                if t < T:
                    cw_ap = cwa[:, t * 64 : (t + 1) * 64]
                    nrow = 77
                elif t == T:  # decode step 0 reuses x = xs[T-1]
                    cw_ap = cwa[:, (T - 1) * 64 : T * 64]
                    nrow = 77
                else:
                    cw_ap = cwd[:]
                    nrow = 65
                for c in range(CH):
                    cs = slice(c * CW, (c + 1) * CW)
                    # attention logits vs both memory banks (96 rows)
                    lg = pp_lg.tile([96, CW], F32, tag="lg")
                    for g in range(3):
                        off = (r * 3 + g) * 96
                        nc.tensor.matmul(
                            lg[:], msk[:, off : off + 96], qb[g][:, cs],
                            start=(g == 0), stop=(g == 2),
                        )
                    ex = sp.tile([96, CW], BF16, tag="ex")
                    nc.scalar.activation(ex[:], lg[:], AF.Exp)
                    # fused mean context (raw) + replicated softmax denominators
                    fu = pp_fu.tile([64, CW], F32, tag="fu")
                    nc.tensor.matmul(fu[:], fmean[:], ex[:], start=True, stop=True)
                    su = pp_su.tile([64, CW], F32, tag="su")
                    nc.tensor.matmul(su[:], fsum[:], ex[:], start=True, stop=True)
                    rt = sp.tile([64, CW], F32, tag="rt")
                    nc.vector.reciprocal_approx_fast(rt[:], su[:])
                    fn = sp.tile([64, CW], BF16, tag="fn")
                    nc.vector.tensor_mul(fn[:], fu[:], rt[:])
                    # gates: zr matmul pair AND its tanh both ran during the
                    # previous step; szr is ready in SBUF
                    szr = szr_pend[c]
                    # both gate sigmoids in ONE 128-row TS; m1 = r*h against
                    # the DMA-maintained h duplicate at partitions 64:128
                    sg = sp.tile([128, CW], BF16, tag="sg")
                    nc.vector.tensor_scalar(
                        sg[:], szr[:], 0.5, 0.5, op0=ALU.mult, op1=ALU.add,
                    )
                    nc.gpsimd.tensor_mul(rhp[64:128, cs], sg[64:128, :], hp[64:128, cs])
                    # candidate: bc + Wx*x + Wc^T (r*h); decode adds the
                    # folded x-feedback term Wx*(ow@h_prev+bo) from hx
                    acc = pp_acc.tile([64, CW], F32, tag="acc")
                    first = True
                    if t > T:
                        nc.tensor.matmul(
                            acc[:], wxd[:], hx[0:65, cs],
                            start=True, stop=False, skip_group_check=True,
                        )
                        first = False
                    nc.tensor.matmul(
                        acc[:], cw_ap, rhp[:, cs],
                        start=first, stop=False, skip_group_check=True,
                    )
                    for k in range(16):
                        n = c * 16 + k
                        nc.tensor.matmul(
                            acc[:, k * 32 : (k + 1) * 32],
                            nsw[:, n * 64 : (n + 1) * 64],
                            fn[:, k * 32 : (k + 1) * 32],
                            start=False, stop=(k == 15), skip_group_check=True,
                        )
                    hcs = sp.tile([64, CW], BF16, tag="hcs")
                    nc.scalar.activation(hcs[:], acc[:], AF.Tanh)
                    # h += z*(hc-h) with z = .5*(s_z+1)
                    dl = sp.tile([64, CW], BF16, tag="dl")
                    nc.vector.tensor_sub(dl[:], hcs[:], hx[0:64, cs])
                    t1 = sp.tile([64, CW], BF16, tag="t1")
                    nc.vector.tensor_mul(t1[:], sg[0:64, :], dl[:])
                    # q(h_new) = Wq^T h_old + Wq^T t1 (split so the h update
                    # itself is off the recurrence-critical path)
                    qp = pp_qp.tile([128, CW], F32, tag="qp")
                    nc.tensor.matmul(
                        qp[row_w : row_w + 32, :], qw2[:], hx[0:65, cs],
                        start=True, stop=False, tile_position=(0, row_w),
                        skip_group_check=True,
                    )
                    nc.tensor.matmul(
                        qp[row_w : row_w + 32, :], qw2[0:64, :], t1[:],
                        start=False, stop=True, tile_position=(0, row_w),
                        skip_group_check=True,
                    )
                    nc.scalar.activation(
                        qb[g_w][row_w : row_w + 32, cs], qp[row_w : row_w + 32, :],
                        AF.Copy,
                    )
                    # zr(t+1) = W^T h_old + W^T t1, issued before the h update
                    if t + 1 < NSTEP:
                        zwn, nrn = zr_w(t + 1)
                        zrt = pp_zr.tile([128, CW], F32, tag="zr", name=f"zr{t+1}_{c}")
                        nc.tensor.matmul(
                            zrt[:], zwn, hx[0:nrn, cs],
                            start=True, stop=False, skip_group_check=True,
                        )
                        nc.tensor.matmul(
                            zrt[:], zwn[0:64, :], t1[:],
                            start=False, stop=True, skip_group_check=True,
                        )
                        szt = szp.tile([128, CW], BF16, tag="szr", name=f"szr{t+1}_{c}")
                        nc.scalar.activation(szt[:], zrt[:], AF.Tanh, scale=0.5)
                        szr_pend[c] = szt
                    # h update off-path on gpsimd, then refresh the h-dup
                    nc.gpsimd.tensor_add(hx[0:64, cs], hx[0:64, cs], t1[:])
                    nc.sync.dma_start(hp[64:128, cs], hx[0:64, cs])
                    if dec:
                        dstep = t - T
                        nc.sync.dma_start(
                            hs_d.ap()[dstep * 64 : (dstep + 1) * 64, cs],
                            hx[0:64, cs],
                        )
    nc.compile()
    return nc


def precompute(inp):
    lm = np.asarray(inp["local_mem"], np.float32)
    gm = np.asarray(inp["global_mem"], np.float32)
    Wq = np.asarray(inp["Wq"], np.float32)
    bq = np.asarray(inp["bq"], np.float32)
    node_emb = np.asarray(inp["node_emb"], np.float32)
    wp = np.asarray(inp["weight_pool"], np.float32)
    Wz = np.asarray(inp["Wz"], np.float32)
    bz = np.asarray(inp["bz"], np.float32)
    Wr = np.asarray(inp["Wr"], np.float32)
    br = np.asarray(inp["br"], np.float32)
    Wc = np.asarray(inp["Wc"], np.float32)
    bc = np.asarray(inp["bc"], np.float32)
    Wo = np.asarray(inp["Wo"], np.float32)
    bo = np.asarray(inp["bo"], np.float32)
    ow = Wo[:, 0]         # [H]
    bo0 = float(bo[0])

    c = {}
    c["nsw_full"] = np.einsum("nd,dfh->nfh", node_emb, wp).astype(np.float32)
    # memory rotation stacks (q-slot j at step r pairs with mem slice (j-r)%S)
    memsl = np.concatenate([lm.transpose(2, 0, 1), gm.transpose(2, 0, 1)], axis=1)  # [P,96,S]
    ms = np.zeros((128, S, 3, 96), np.float32)
    for r in range(S):
        for g in range(3):
            for i in range(4):
                s = (4 * g + i - r) % S
                ms[32 * i : 32 * (i + 1), r, g, :] = memsl[:, :, s]
    c["memstack"] = ms.reshape(128, S * 3 * 96)
    lmean, gmean = lm.mean(axis=1), gm.mean(axis=1)
    fme = np.zeros((96, 64), np.float32)
    fme[:ML, :P] = lmean
    fme[ML:, P : 2 * P] = gmean
    c["fmean"] = fme
    fsu = np.zeros((96, 64), np.float32)
    fsu[:ML, :P] = 1.0
    fsu[ML:, P : 2 * P] = 1.0
    c["fsum"] = fsu

    # encode z|r combined: rows 0:64 [Wz|Wr], row 64 [bz|br], row 65+t [Wxz|Wxr]
    zrw = np.zeros((77, T, 128), np.float32)
    zrw[:H, :, :H] = Wz[1:][:, None, :]
    zrw[:H, :, H:] = Wr[1:][:, None, :]
    zrw[H, :, :H] = bz[None, :]
    zrw[H, :, H:] = br[None, :]
    for t in range(T):
        zrw[H + 1 + t, t, :H] = Wz[0]
        zrw[H + 1 + t, t, H:] = Wr[0]
    c["zrwa"] = zrw.reshape(77, T * 128)
    # decode z|r: x = y_prev folded as rank-1 update (x = ow@h_prev + bo)
    zrd = np.zeros((65, 128), np.float32)
    zrd[:H, :H] = Wz[1:] + np.outer(ow, Wz[0])
    zrd[:H, H:] = Wr[1:] + np.outer(ow, Wr[0])
    zrd[H, :H] = bz + bo0 * Wz[0]
    zrd[H, H:] = br + bo0 * Wr[0]
    c["zrwd"] = zrd
    # candidate m1-stream weights: row 0 bc, rows 1+t Wcx, rows 64:128 Wc
    cwm = np.zeros((128, T, 64), np.float32)
    cwm[0, :, :] = bc[None, :]
    for t in range(T):
        cwm[1 + t, t, :] = Wc[0]
    cwm[64:, :, :] = Wc[1:][:, None, :]
    c["cwa"] = cwm.reshape(128, T * 64)
    cwdm = np.zeros((128, 64), np.float32)
    cwdm[0] = bc
    cwdm[64:] = Wc[1:]
    c["cwd"] = cwdm
    wxdm = np.zeros((65, 64), np.float32)
    wxdm[:H] = np.outer(ow, Wc[0])
    wxdm[H] = bo0 * Wc[0]
    c["wxd"] = wxdm
    qw2 = np.zeros((65, 32), np.float32)
    qw2[:H] = Wq
    qw2[H] = bq
    c["qw2"] = qw2
    c["qbinit"] = np.broadcast_to(np.tile(bq, 4).reshape(128, 1), (128, NB)).copy()
    c["ow"] = ow
    c["bo0"] = bo0
    return c


def _bf16(a):
    import ml_dtypes
    return np.ascontiguousarray(a).astype(ml_dtypes.bfloat16)


def make_in_maps(inp):
    c = precompute(inp)
    src = np.asarray(inp["source"], np.float32)
    shared = {
        "memstack": _bf16(c["memstack"]), "fmean": _bf16(c["fmean"]),
        "fsum": _bf16(c["fsum"]), "zrwa": _bf16(c["zrwa"]), "zrwd": _bf16(c["zrwd"]),
        "cwa": _bf16(c["cwa"]), "cwd": _bf16(c["cwd"]), "wxd": _bf16(c["wxd"]),
        "qw2": _bf16(c["qw2"]), "qbinit": _bf16(c["qbinit"]),
    }
    in_maps = []
    for core in range(NCORES):
        nodes = slice(core * NL, (core + 1) * NL)
        xs = src[:, :, nodes, 0].transpose(1, 2, 0).reshape(T, NB)
        hxi = np.zeros((77, NB), np.float32)
        hxi[64] = 1.0
        hxi[65:77] = xs
        rhi = np.zeros((128, NB), np.float32)
        rhi[0] = 1.0
        rhi[1:13] = xs
        nswc = _bf16(c["nsw_full"][nodes].transpose(1, 0, 2).reshape(64, NL * 64))
        in_maps.append(dict(shared, hxinit=_bf16(hxi), rhpinit=_bf16(rhi), nsw=nswc))
    return in_maps


def assemble(results, ow, bo0):
    out = np.zeros((B, HORIZON, N, OUT), np.float32)
    for core in range(NCORES):
        nodes = slice(core * NL, (core + 1) * NL)
        hs = np.asarray(results[core]["hsave"], np.float32)  # [HORIZON*64, NB]
        hs = hs.reshape(HORIZON, 64, NL, B)
        ys = np.einsum("k,dknb->dnb", ow, hs) + bo0          # [HORIZON, NL, B]
        out[:, :, nodes, 0] = ys.transpose(2, 0, 1)
    return out


_NC_CACHE = {}


def kernel(**inputs):
    if "nc" not in _NC_CACHE:
        _NC_CACHE["nc"] = build_nc()
    nc = _NC_CACHE["nc"]
    c_ow = np.asarray(inputs["Wo"], np.float32)[:, 0]
    c_bo = float(np.asarray(inputs["bo"], np.float32)[0])
    in_maps = make_in_maps(inputs)
    res = bass_utils.run_bass_kernel_spmd(nc, in_maps, core_ids=list(range(NCORES)))
    return assemble(res.results, c_ow, c_bo)


# revision 50
# speedup vs baseline: 1.0367x; 1.0367x over previous
"""Trainium2 Bass kernel for nn_H_DYNA_42348377538865 (scatter_memory GRU + memory attention).

Self-contained: shards node dim N=512 across 8 NeuronCores (64 nodes/core),
runs a fully-unrolled 24-step recurrence per core, gathers on host.

Layout: feature-on-partitions, (node, batch) on free dim (col = n_local*32 + b,
NB=2048 cols/core, 4 column chunks of 512). Design notes (~2.2x vs the v1
scatter kernel's 634us; TimelineSim 291us, all four engines ~80% busy):
  - single activation table: sigmoid computed as 0.5+0.5*tanh(x/2) so the
    Scalar engine only ever uses {Exp, Tanh, Copy/Identity} -> zero
    LoadActFuncSet swaps in steady state (each swap costs 1283ns; the v1
    kernel spent 164us on them).
  - biases folded into matmuls via a const-1.0 row of the state tiles;
    encode inputs x_0..x_11 pre-staged as extra state rows with per-step
    weight rows selecting them (no per-step x DMA, no step barriers).
  - decode x-feedback (x_t = y_{t-1}) folded algebraically into the decode
    weights (rank-1 update W + ow@Wx^T; candidate path gets an extra wxd
    matmul); y itself is never computed on device - decode h states are
    DMA'd out and y = ow@h + bo applied on host.
  - gates: ONE combined [*,128] z|r matmul + ONE tanh. The zr matmul for
    step t+1 and its tanh are issued during step t as a split pair
    (W^T h_old + W^T t1, exploiting linearity), and q(h_new) is likewise
    Wq^T h_old + Wq^T t1, so the in-place h update runs OFF the recurrence
    critical path on the otherwise-idle GPSIMD engine.
  - engine balance (each ~240us busy): ACT: exp, gate-tanh, cand-tanh,
    q-copy; DVE: recip, softmax-normalize mul, r/z gate affines (4x-mode
    tensor_scalar, the r one partition-crossed 64->0), hc-h sub, z*dl mul;
    GPSIMD: r*h product and the h update (both off the critical cycle);
    PE: 10-11 streams/chunk.
  - hardware rules learned (see test scripts): multi-tensor DVE/GPSIMD ops
    need all operands at the same start partition; single-input ops
    (activation, tensor_copy, tensor_scalar) may cross partition offsets,
    but reciprocal_approx_fast silently NaNs on crossed reads; tensor_tensor
    divide and fused scalar_tensor_tensor on GPSIMD are rejected/crash.
  - rolling q-cache as v1: 12 slots in 3x[128,NB] tiles; slot j pairs with
    memory slice s=(j-t)%12 via 12 precomputed rotation stacks.
"""
import numpy as np
import sys

for _p in ("/opt/trn_rl_repo",):
    if _p not in sys.path:
        sys.path.append(_p)

import concourse.bass as bass
import concourse.bacc as bacc
import concourse.mybir as mybir
import concourse.tile as tile
from concourse import bass_utils

B, T, HORIZON, N = 32, 12, 12, 512
IN, OUT, H, P = 1, 1, 64, 32
S, ML, MG, DE = 12, 64, 32, 10
NCORES = 8
NL = N // NCORES        # 64
NB = NL * B             # 2048
NSTEP = T + HORIZON     # 24
CH = 4                  # column chunks
CW = NB // CH           # 512

F32 = mybir.dt.float32
BF16 = mybir.dt.bfloat16
AF = mybir.ActivationFunctionType
ALU = mybir.AluOpType


def build_nc():
    nc = bacc.Bacc("TRN2", target_bir_lowering=False, debug=False)
    d = {}
    d["hxinit"] = nc.dram_tensor("hxinit", [77, NB], BF16, kind="ExternalInput")
    d["qbinit"] = nc.dram_tensor("qbinit", [128, NB], BF16, kind="ExternalInput")
    d["memstack"] = nc.dram_tensor("memstack", [128, S * 3 * 96], BF16, kind="ExternalInput")
    d["nsw"] = nc.dram_tensor("nsw", [64, NL * 64], BF16, kind="ExternalInput")
    d["fmean"] = nc.dram_tensor("fmean", [96, 64], BF16, kind="ExternalInput")
    d["fsum"] = nc.dram_tensor("fsum", [96, 64], BF16, kind="ExternalInput")
    d["zrwa"] = nc.dram_tensor("zrwa", [77, T * 128], BF16, kind="ExternalInput")
    d["zrwd"] = nc.dram_tensor("zrwd", [65, 128], BF16, kind="ExternalInput")
    d["cwa"] = nc.dram_tensor("cwa", [77, T * 64], BF16, kind="ExternalInput")
    d["cwd"] = nc.dram_tensor("cwd", [65, 64], BF16, kind="ExternalInput")
    d["wxd"] = nc.dram_tensor("wxd", [65, 64], BF16, kind="ExternalInput")
    d["qw2"] = nc.dram_tensor("qw2", [65, 32], BF16, kind="ExternalInput")
    hs_d = nc.dram_tensor("hsave", [HORIZON * 64, NB], BF16, kind="ExternalOutput")

    with tile.TileContext(nc) as tc:
        with (
            tc.tile_pool(name="consts", bufs=1) as cp,
            tc.tile_pool(name="sp", bufs=8) as sp,
            tc.tile_pool(name="szp", bufs=12) as szp,
            tc.tile_pool(name="pp_lg", bufs=2, space="PSUM") as pp_lg,
            tc.tile_pool(name="pp_fu", bufs=1, space="PSUM") as pp_fu,
            tc.tile_pool(name="pp_su", bufs=1, space="PSUM") as pp_su,
            tc.tile_pool(name="pp_zr", bufs=2, space="PSUM") as pp_zr,
            tc.tile_pool(name="pp_acc", bufs=1, space="PSUM") as pp_acc,
            tc.tile_pool(name="pp_qp", bufs=1, space="PSUM") as pp_qp,
        ):
            # critical-path loads first: initial state, zr weights, q-cache
            # and memstack gate step 0; the rest arrives under compute
            hx = cp.tile([77, NB], BF16)
            nc.sync.dma_start(hx[:], d["hxinit"].ap())
            zrwa = cp.tile([77, T * 128], BF16)
            nc.sync.dma_start(zrwa[:], d["zrwa"].ap())
            qb = []
            for g in range(3):
                q = cp.tile([128, NB], BF16, name=f"qb{g}")
                nc.sync.dma_start(q[:], d["qbinit"].ap())
                qb.append(q)
            msk = cp.tile([128, S * 3 * 96], BF16)
            nc.sync.dma_start(msk[:], d["memstack"].ap())
            fmean = cp.tile([96, 64], BF16)
            nc.sync.dma_start(fmean[:], d["fmean"].ap())
            fsum = cp.tile([96, 64], BF16)
            nc.sync.dma_start(fsum[:], d["fsum"].ap())
            nsw = cp.tile([64, NL * 64], BF16)
            nc.sync.dma_start(nsw[:], d["nsw"].ap())
            rhp = cp.tile([77, NB], BF16)
            nc.sync.dma_start(rhp[:], d["hxinit"].ap())
            cwa = cp.tile([77, T * 64], BF16)
            nc.sync.dma_start(cwa[:], d["cwa"].ap())
            qw2 = cp.tile([65, 32], BF16)
            nc.sync.dma_start(qw2[:], d["qw2"].ap())
            zrwd = cp.tile([65, 128], BF16)
            nc.sync.dma_start(zrwd[:], d["zrwd"].ap())
            cwd = cp.tile([65, 64], BF16)
            nc.sync.dma_start(cwd[:], d["cwd"].ap())
            wxd = cp.tile([65, 64], BF16)
            nc.sync.dma_start(wxd[:], d["wxd"].ap())

            def zr_w(t):
                # (zr weights AP, stream row count) for step t
                if t < T:
                    return zrwa[:, t * 128 : (t + 1) * 128], 77
                if t == T:  # decode step 0 reuses x = xs[T-1]
                    return zrwa[:, (T - 1) * 128 : T * 128], 77
                return zrwd[:], 65

            # zr(0)+tanh for all chunks from the initial state
            szr_pend = []
            for c in range(CH):
                zw0, nr0 = zr_w(0)
                zrt = pp_zr.tile([128, CW], F32, tag="zr", name=f"zr0_{c}")
                nc.tensor.matmul(
                    zrt[:], zw0, hx[0:nr0, c * CW : (c + 1) * CW],
                    start=True, stop=True,
                )
                szt = szp.tile([128, CW], BF16, tag="szr", name=f"szr0_{c}")
                nc.scalar.activation(szt[:], zrt[:], AF.Tanh, scale=0.5)
                szr_pend.append(szt)

            for t in range(NSTEP):
                r = t % S
                j = t % S
                g_w, row_w = j // 4, (j % 4) * 32
                dec = t >= T
                if t < T:
                    cw_ap = cwa[:, t * 64 : (t + 1) * 64]
                    nrow = 77
                elif t == T:  # decode step 0 reuses x = xs[T-1]
                    cw_ap = cwa[:, (T - 1) * 64 : T * 64]
                    nrow = 77
                else:
                    cw_ap = cwd[:]
                    nrow = 65
                for c in range(CH):
                    cs = slice(c * CW, (c + 1) * CW)
                    # attention logits vs both memory banks (96 rows)
                    lg = pp_lg.tile([96, CW], F32, tag="lg")
                    for g in range(3):
                        off = (r * 3 + g) * 96
                        nc.tensor.matmul(
                            lg[:], msk[:, off : off + 96], qb[g][:, cs],
                            start=(g == 0), stop=(g == 2),
                        )
                    ex = sp.tile([96, CW], BF16, tag="ex")
                    nc.scalar.activation(ex[:], lg[:], AF.Exp)
                    # fused mean context (raw) + replicated softmax denominators
                    fu = pp_fu.tile([64, CW], F32, tag="fu")
                    nc.tensor.matmul(fu[:], fmean[:], ex[:], start=True, stop=True)
                    su = pp_su.tile([64, CW], F32, tag="su")
                    nc.tensor.matmul(su[:], fsum[:], ex[:], start=True, stop=True)
                    rt = sp.tile([64, CW], F32, tag="rt")
                    nc.vector.reciprocal_approx_fast(rt[:], su[:])
                    fn = sp.tile([64, CW], BF16, tag="fn")
                    nc.vector.tensor_mul(fn[:], fu[:], rt[:])
                    # gates: zr matmul pair AND its tanh both ran during the
                    # previous step; szr is ready in SBUF
                    szr = szr_pend[c]
                    # r gate (real sigmoid, cross-partition TS) and m1 = r*h
                    rg = sp.tile([64, CW], BF16, tag="rg")
                    nc.vector.tensor_scalar(
                        rg[:], szr[64:128, :], 0.5, 0.5,
                        op0=ALU.mult, op1=ALU.add,
                    )
                    nc.gpsimd.tensor_mul(rhp[0:64, cs], rg[:], hx[0:64, cs])
                    # candidate: bc + Wx*x + Wc^T (r*h); decode adds the
                    # folded x-feedback term Wx*(ow@h_prev+bo) from hx
                    acc = pp_acc.tile([64, CW], F32, tag="acc")
                    first = True
                    if t > T:
                        nc.tensor.matmul(
                            acc[:], wxd[:], hx[0:65, cs],
                            start=True, stop=False, skip_group_check=True,
                        )
                        first = False
                    nc.tensor.matmul(
                        acc[:], cw_ap, rhp[0:nrow, cs],
                        start=first, stop=False, skip_group_check=True,
                    )
                    for k in range(16):
                        n = c * 16 + k
                        nc.tensor.matmul(
                            acc[:, k * 32 : (k + 1) * 32],
                            nsw[:, n * 64 : (n + 1) * 64],
                            fn[:, k * 32 : (k + 1) * 32],
                            start=False, stop=(k == 15), skip_group_check=True,
                        )
                    hcs = sp.tile([64, CW], BF16, tag="hcs")
                    nc.scalar.activation(hcs[:], acc[:], AF.Tanh)
                    # h += z*(hc-h) with z = .5*(s_z+1)
                    zg = sp.tile([64, CW], BF16, tag="zg")
                    nc.vector.tensor_scalar(
                        zg[:], szr[0:64, :], 0.5, 0.5, op0=ALU.mult, op1=ALU.add,
                    )
                    dl = sp.tile([64, CW], BF16, tag="dl")
                    nc.vector.tensor_sub(dl[:], hcs[:], hx[0:64, cs])
                    t1 = sp.tile([64, CW], BF16, tag="t1")
                    nc.vector.tensor_mul(t1[:], zg[:], dl[:])
                    # q(h_new) = Wq^T h_old + Wq^T t1 (split so the h update
                    # itself is off the recurrence-critical path)
                    qp = pp_qp.tile([128, CW], F32, tag="qp")
                    nc.tensor.matmul(
                        qp[row_w : row_w + 32, :], qw2[:], hx[0:65, cs],
                        start=True, stop=False, tile_position=(0, row_w),
                        skip_group_check=True,
                    )
                    nc.tensor.matmul(
                        qp[row_w : row_w + 32, :], qw2[0:64, :], t1[:],
                        start=False, stop=True, tile_position=(0, row_w),
                        skip_group_check=True,
                    )
                    nc.scalar.activation(
                        qb[g_w][row_w : row_w + 32, cs], qp[row_w : row_w + 32, :],
                        AF.Copy,
                    )
                    # zr(t+1) = W^T h_old + W^T t1, issued before the h update
                    if t + 1 < NSTEP:
                        zwn, nrn = zr_w(t + 1)
                        zrt = pp_zr.tile([128, CW], F32, tag="zr", name=f"zr{t+1}_{c}")
                        nc.tensor.matmul(
                            zrt[:], zwn, hx[0:nrn, cs],
                            start=True, stop=False, skip_group_check=True,
                        )
                        nc.tensor.matmul(
                            zrt[:], zwn[0:64, :], t1[:],
                            start=False, stop=True, skip_group_check=True,
                        )
                        szt = szp.tile([128, CW], BF16, tag="szr", name=f"szr{t+1}_{c}")
                        nc.scalar.activation(szt[:], zrt[:], AF.Tanh, scale=0.5)
                        szr_pend[c] = szt
                    # h update off-path on gpsimd
                    nc.gpsimd.tensor_add(hx[0:64, cs], hx[0:64, cs], t1[:])
                    if dec:
                        dstep = t - T
                        nc.sync.dma_start(
                            hs_d.ap()[dstep * 64 : (dstep + 1) * 64, cs],
                            hx[0:64, cs],
                        )
    nc.compile()
    return nc


def precompute(inp):
    lm = np.asarray(inp["local_mem"], np.float32)
    gm = np.asarray(inp["global_mem"], np.float32)
    Wq = np.asarray(inp["Wq"], np.float32)
    bq = np.asarray(inp["bq"], np.float32)
    node_emb = np.asarray(inp["node_emb"], np.float32)
    wp = np.asarray(inp["weight_pool"], np.float32)
    Wz = np.asarray(inp["Wz"], np.float32)
    bz = np.asarray(inp["bz"], np.float32)
    Wr = np.asarray(inp["Wr"], np.float32)
    br = np.asarray(inp["br"], np.float32)
    Wc = np.asarray(inp["Wc"], np.float32)
    bc = np.asarray(inp["bc"], np.float32)
    Wo = np.asarray(inp["Wo"], np.float32)
    bo = np.asarray(inp["bo"], np.float32)
    ow = Wo[:, 0]         # [H]
    bo0 = float(bo[0])

    c = {}
    c["nsw_full"] = np.einsum("nd,dfh->nfh", node_emb, wp).astype(np.float32)
    # memory rotation stacks (q-slot j at step r pairs with mem slice (j-r)%S)
    memsl = np.concatenate([lm.transpose(2, 0, 1), gm.transpose(2, 0, 1)], axis=1)  # [P,96,S]
    ms = np.zeros((128, S, 3, 96), np.float32)
    for r in range(S):
        for g in range(3):
            for i in range(4):
                s = (4 * g + i - r) % S
                ms[32 * i : 32 * (i + 1), r, g, :] = memsl[:, :, s]
    c["memstack"] = ms.reshape(128, S * 3 * 96)
    lmean, gmean = lm.mean(axis=1), gm.mean(axis=1)
    fme = np.zeros((96, 64), np.float32)
    fme[:ML, :P] = lmean
    fme[ML:, P : 2 * P] = gmean
    c["fmean"] = fme
    fsu = np.zeros((96, 64), np.float32)
    fsu[:ML, :P] = 1.0
    fsu[ML:, P : 2 * P] = 1.0
    c["fsum"] = fsu

    # encode z|r combined: rows 0:64 [Wz|Wr], row 64 [bz|br], row 65+t [Wxz|Wxr]
    zrw = np.zeros((77, T, 128), np.float32)
    zrw[:H, :, :H] = Wz[1:][:, None, :]
    zrw[:H, :, H:] = Wr[1:][:, None, :]
    zrw[H, :, :H] = bz[None, :]
    zrw[H, :, H:] = br[None, :]
    for t in range(T):
        zrw[H + 1 + t, t, :H] = Wz[0]
        zrw[H + 1 + t, t, H:] = Wr[0]
    c["zrwa"] = zrw.reshape(77, T * 128)
    # decode z|r: x = y_prev folded as rank-1 update (x = ow@h_prev + bo)
    zrd = np.zeros((65, 128), np.float32)
    zrd[:H, :H] = Wz[1:] + np.outer(ow, Wz[0])
    zrd[:H, H:] = Wr[1:] + np.outer(ow, Wr[0])
    zrd[H, :H] = bz + bo0 * Wz[0]
    zrd[H, H:] = br + bo0 * Wr[0]
    c["zrwd"] = zrd
    # candidate m1-stream weights: rows 0:64 Wc, row 64 bc, row 65+t Wcx
    cwm = np.zeros((77, T, 64), np.float32)
    cwm[:H, :, :] = Wc[1:][:, None, :]
    cwm[H, :, :] = bc[None, :]
    for t in range(T):
        cwm[H + 1 + t, t, :] = Wc[0]
    c["cwa"] = cwm.reshape(77, T * 64)
    cwdm = np.zeros((65, 64), np.float32)
    cwdm[:H] = Wc[1:]
    cwdm[H] = bc
    c["cwd"] = cwdm
    wxdm = np.zeros((65, 64), np.float32)
    wxdm[:H] = np.outer(ow, Wc[0])
    wxdm[H] = bo0 * Wc[0]
    c["wxd"] = wxdm
    qw2 = np.zeros((65, 32), np.float32)
    qw2[:H] = Wq
    qw2[H] = bq
    c["qw2"] = qw2
    c["qbinit"] = np.broadcast_to(np.tile(bq, 4).reshape(128, 1), (128, NB)).copy()
    c["ow"] = ow
    c["bo0"] = bo0
    return c


def _bf16(a):
    import ml_dtypes
    return np.ascontiguousarray(a).astype(ml_dtypes.bfloat16)


def make_in_maps(inp):
    c = precompute(inp)
    src = np.asarray(inp["source"], np.float32)
    shared = {
        "memstack": _bf16(c["memstack"]), "fmean": _bf16(c["fmean"]),
        "fsum": _bf16(c["fsum"]), "zrwa": _bf16(c["zrwa"]), "zrwd": _bf16(c["zrwd"]),
        "cwa": _bf16(c["cwa"]), "cwd": _bf16(c["cwd"]), "wxd": _bf16(c["wxd"]),
        "qw2": _bf16(c["qw2"]), "qbinit": _bf16(c["qbinit"]),
    }
    in_maps = []
    for core in range(NCORES):
        nodes = slice(core * NL, (core + 1) * NL)
        xs = src[:, :, nodes, 0].transpose(1, 2, 0).reshape(T, NB)
        hxi = np.zeros((77, NB), np.float32)
        hxi[64] = 1.0
        hxi[65:77] = xs
        nswc = _bf16(c["nsw_full"][nodes].transpose(1, 0, 2).reshape(64, NL * 64))
        in_maps.append(dict(shared, hxinit=_bf16(hxi), nsw=nswc))
    return in_maps


def assemble(results, ow, bo0):
    out = np.zeros((B, HORIZON, N, OUT), np.float32)
    for core in range(NCORES):
        nodes = slice(core * NL, (core + 1) * NL)
        hs = np.asarray(results[core]["hsave"], np.float32)  # [HORIZON*64, NB]
        hs = hs.reshape(HORIZON, 64, NL, B)
        ys = np.einsum("k,dknb->dnb", ow, hs) + bo0          # [HORIZON, NL, B]
        out[:, :, nodes, 0] = ys.transpose(2, 0, 1)
    return out


_NC_CACHE = {}


def kernel(**inputs):
    if "nc" not in _NC_CACHE:
        _NC_CACHE["nc"] = build_nc()
    nc = _NC_CACHE["nc"]
    c_ow = np.asarray(inputs["Wo"], np.float32)[:, 0]
    c_bo = float(np.asarray(inputs["bo"], np.float32)[0])
    in_maps = make_in_maps(inputs)
    res = bass_utils.run_bass_kernel_spmd(nc, in_maps, core_ids=list(range(NCORES)))
    return assemble(res.results, c_ow, c_bo)
